# revision 6
# baseline (speedup 1.0000x reference)
"""DeepSeek-V3-style MoE layer on 8 Trainium2 NeuronCores.

Strategy (expert-parallel, host-routed):
  - Gate (sigmoid + group-limited top-k) is computed on host with jax/CPU,
    mirroring the reference ops exactly so expert selection is bit-identical.
  - Tokens are gathered per expert on host (capacity-padded), transposed to
    [DIM, C] so the device kernel is a pure grouped GEMM:
        hT = silu(W1 @ xgT) * (W3 @ xgT);  ygT = W2 @ hT
    Each of the 8 cores owns 4 of the 32 experts (expert parallelism) plus a
    1/8 token-slice of the shared SwiGLU expert (data parallelism).
  - Matmuls run as float32r (TF32) at 1 cycle/row; inputs are pre-rounded to
    TF32 on host; accumulation is fp32 in PSUM.
  - Host applies the routed combine weights during the scatter-add epilogue.
"""

import math
import numpy as np

DIM = 2048
INTER = 1408
N_EXPERTS = 32
TOPK = 6
N_GROUPS = 8
TOPK_GROUPS = 4
ROUTE_SCALE = 2.5
SHARED_INTER = 2816
T = 8192

NCORES = 8
ELOC = N_EXPERTS // NCORES          # 4 experts per core
TS = T // NCORES                    # 1024 shared-expert tokens per core
KT = DIM // 128                     # 16 contraction tiles (dim)
MT = INTER // 128                   # 11 inter tiles
SMT = SHARED_INTER // 128           # 22 shared inter tiles

_prog_cache = {}


def _round_tf32(a: np.ndarray) -> np.ndarray:
    """Round fp32 to TF32 (10-bit mantissa), round-to-nearest-even."""
    b = np.ascontiguousarray(a, dtype=np.float32).view(np.uint32)
    b = b + 0x0FFF + ((b >> 13) & 1)
    b &= np.uint32(0xFFFFE000)
    return b.view(np.float32)


def _gate_host(x, gate_w):
    """Bit-identical copy of the reference gate, forced onto jax CPU."""
    import jax
    import jax.numpy as jnp

    cpu = jax.devices("cpu")[0]
    with jax.default_device(cpu):
        xj = jnp.asarray(x)
        gj = jnp.asarray(gate_w)
        scores = jax.nn.sigmoid(xj @ gj.T)
        original = scores
        sg = scores.reshape(x.shape[0], N_GROUPS, -1)
        group_scores = sg.max(axis=-1)
        _, gidx = jax.lax.top_k(group_scores, TOPK_GROUPS)
        gmask = jnp.zeros((x.shape[0], N_GROUPS), bool).at[
            jnp.arange(x.shape[0])[:, None], gidx].set(True)
        masked = jnp.where(gmask[:, :, None], sg, 0.0).reshape(x.shape[0], -1)
        _, idx = jax.lax.top_k(masked, TOPK)
        w = jnp.take_along_axis(original, idx, axis=1)
        w = w / w.sum(axis=-1, keepdims=True)
        w = w * ROUTE_SCALE
        return np.asarray(w, dtype=np.float32), np.asarray(idx, dtype=np.int32)


def _chunks(width):
    """Split width into matmul moving-dim chunks <=512, each >=256."""
    out = []
    rem = width
    while rem > 0:
        if rem > 512:
            w = 512 if rem - 512 >= 256 else 384
        else:
            w = rem
        out.append(w)
        rem -= w
    return out


def _build_program(C, n_parts, eloc=ELOC, with_shared=True, with_routed=True):
    import concourse.tile as tile
    from concourse import bacc, mybir

    f32 = mybir.dt.float32
    f32r = mybir.dt.float32r
    AF = mybir.ActivationFunctionType

    CH = C // n_parts
    r_chunks = _chunks(CH)           # routed: chunk widths within a part
    s_chunks = [256, 256]            # shared: 512-token halves, 256-wide chunks

    nc = bacc.Bacc(None, target_bir_lowering=False)

    xg_d = nc.dram_tensor("xg", [eloc, KT, 128, C], f32r, kind="ExternalInput")
    w1_d = nc.dram_tensor("w1t", [eloc, MT, 128, KT * 128], f32r, kind="ExternalInput")
    w3_d = nc.dram_tensor("w3t", [eloc, MT, 128, KT * 128], f32r, kind="ExternalInput")
    w2_d = nc.dram_tensor("w2t", [eloc, KT, 128, MT * 128], f32r, kind="ExternalInput")
    xs_d = nc.dram_tensor("xs", [KT, 128, TS], f32r, kind="ExternalInput")
    sw1_d = nc.dram_tensor("sw1t", [SMT, 128, KT * 128], f32r, kind="ExternalInput")
    sw3_d = nc.dram_tensor("sw3t", [SMT, 128, KT * 128], f32r, kind="ExternalInput")
    sw2_d = nc.dram_tensor("sw2t", [KT, 128, SMT * 128], f32r, kind="ExternalInput")
    yg_d = nc.dram_tensor("yg", [eloc, KT, 128, C], f32, kind="ExternalOutput")
    zs_d = nc.dram_tensor("zs", [KT, 128, TS], f32, kind="ExternalOutput")

    with tile.TileContext(nc) as tc:
        with tc.tile_pool(name="main", bufs=1) as mp, \
             tc.tile_pool(name="psum", bufs=1, space="PSUM") as pp:

            def mlp_part(x_tiles, n_mt, w1_src, w3_src, w2_src, chunk_ws,
                         h_tag, h_bufs, y_sink):
                """One SwiGLU MLP over a token-part already resident in SBUF.

                x_tiles: KT tiles [128, W] (f32r); W = sum(chunk_ws)
                n_mt: inter tiles (11 routed / 22 shared)
                w1_src/w3_src: DRAM AP indexable [m] -> [KT,128,128]
                w2_src: DRAM AP indexable [m2] -> [n_mt,128,128]
                y_sink(m2, off, width, sbuf_tile): emit output DMA
                """
                W = sum(chunk_ws)
                h_tiles = []
                for m in range(n_mt):
                    w1_t = mp.tile([128, KT * 128], f32r, tag="w1", bufs=2, name=f"w1_{m}")
                    nc.sync.dma_start(out=w1_t, in_=w1_src[m])
                    w3_t = mp.tile([128, KT * 128], f32r, tag="w3", bufs=2, name=f"w3_{m}")
                    nc.sync.dma_start(out=w3_t, in_=w3_src[m])
                    h_t = mp.tile([128, W], f32r, tag=h_tag, bufs=h_bufs, name=f"h_{m}")
                    h_tiles.append(h_t)
                    off = 0
                    for cw in chunk_ws:
                        pa = pp.tile([128, cw], f32, tag="pa", bufs=2, name="pa")
                        pb = pp.tile([128, cw], f32, tag="pb", bufs=2, name="pb")
                        for k in range(KT):
                            nc.tensor.matmul(
                                pa, lhsT=w1_t[:, k * 128:(k + 1) * 128],
                                rhs=x_tiles[k][:, off:off + cw],
                                start=(k == 0), stop=(k == KT - 1))
                        for k in range(KT):
                            nc.tensor.matmul(
                                pb, lhsT=w3_t[:, k * 128:(k + 1) * 128],
                                rhs=x_tiles[k][:, off:off + cw],
                                start=(k == 0), stop=(k == KT - 1))
                        sil = mp.tile([128, cw], f32, tag="sil", bufs=2, name="sil")
                        nc.scalar.activation(out=sil, in_=pa, func=AF.Silu)
                        nc.vector.tensor_mul(h_t[:, off:off + cw], sil, pb)
                        off += cw
                for m2 in range(KT):
                    w2_t = mp.tile([128, n_mt * 128], f32r,
                                   tag=("w2" if n_mt == MT else "sw2"), bufs=2,
                                   name=f"w2_{m2}")
                    nc.sync.dma_start(out=w2_t, in_=w2_src[m2])
                    off = 0
                    for cw in chunk_ws:
                        py = pp.tile([128, cw], f32, tag="py", bufs=2, name="py")
                        for k2 in range(n_mt):
                            nc.tensor.matmul(
                                py, lhsT=w2_t[:, k2 * 128:(k2 + 1) * 128],
                                rhs=h_tiles[k2][:, off:off + cw],
                                start=(k2 == 0), stop=(k2 == n_mt - 1))
                        yo = mp.tile([128, cw], f32, tag="yo", bufs=2, name="yo")
                        nc.vector.tensor_copy(yo, py)
                        y_sink(m2, off, cw, yo)
                        off += cw

            # ---- routed experts ----
            for e in range(eloc if with_routed else 0):
                for part in range(n_parts):
                    base = part * CH
                    x_tiles = []
                    for k in range(KT):
                        xt = mp.tile([128, CH], f32r, tag="xg", bufs=KT, name=f"xg_{k}")
                        nc.sync.dma_start(out=xt, in_=xg_d[e, k, :, base:base + CH])
                        x_tiles.append(xt)

                    def y_sink(m2, off, cw, yo, e=e, base=base):
                        nc.sync.dma_start(
                            out=yg_d[e, m2, :, base + off:base + off + cw], in_=yo)

                    mlp_part(x_tiles, MT, w1_d[e], w3_d[e], w2_d[e], r_chunks,
                             "h", MT, y_sink)

            # ---- shared expert (1/8 token slice), two 512-token halves ----
            for part in range(2 if with_shared else 0):
                base = part * 512
                x_tiles = []
                for k in range(KT):
                    xt = mp.tile([128, 512], f32r, tag="xg", bufs=KT, name=f"xs_{k}")
                    nc.sync.dma_start(out=xt, in_=xs_d[k, :, base:base + 512])
                    x_tiles.append(xt)

                def z_sink(m2, off, cw, yo, base=base):
                    nc.sync.dma_start(
                        out=zs_d[m2, :, base + off:base + off + cw], in_=yo)

                mlp_part(x_tiles, SMT, sw1_d, sw3_d, sw2_d, s_chunks,
                         "hs", SMT, z_sink)

    nc.finalize()
    return nc


def _get_program(C, n_parts):
    key = (C, n_parts)
    if key not in _prog_cache:
        _prog_cache[key] = _build_program(C, n_parts)
    return _prog_cache[key]


def kernel(x, gate_w, w1, w2, w3, sw1, sw2, sw3):
    from concourse.bass_utils import run_bass_kernel_spmd

    x = np.ascontiguousarray(np.asarray(x, dtype=np.float32))
    gate_w = np.asarray(gate_w, dtype=np.float32)
    w1 = np.asarray(w1, dtype=np.float32)
    w2 = np.asarray(w2, dtype=np.float32)
    w3 = np.asarray(w3, dtype=np.float32)
    sw1 = np.asarray(sw1, dtype=np.float32)
    sw2 = np.asarray(sw2, dtype=np.float32)
    sw3 = np.asarray(sw3, dtype=np.float32)

    # ---- host routing (bit-identical to reference gate) ----
    weights, idx = _gate_host(x, gate_w)

    flat_e = idx.ravel()
    flat_tok = np.repeat(np.arange(T, dtype=np.int64), TOPK)
    flat_w = weights.ravel()
    order = np.argsort(flat_e, kind="stable")
    sorted_tok = flat_tok[order]
    sorted_w = flat_w[order]
    counts = np.bincount(flat_e, minlength=N_EXPERTS)
    offs = np.concatenate([[0], np.cumsum(counts)])

    C = max(1664, int(math.ceil(counts.max() / 256.0)) * 256)
    # SBUF budget check: fall back to quarter-parts for very large capacity
    n_parts = 2
    if (KT + MT) * (C // n_parts) * 4 > 92 * 1024:
        n_parts = 4
        C = int(math.ceil(C / 512.0)) * 512

    # ---- host data prep (TF32 rounding + layouts) ----
    x_r = _round_tf32(x)
    xT_r = np.ascontiguousarray(x_r.T)            # [DIM, T]

    tok_pad = np.zeros((N_EXPERTS, C), dtype=np.int64)
    for e in range(N_EXPERTS):
        te = sorted_tok[offs[e]:offs[e + 1]]
        tok_pad[e, :len(te)] = te

    def wtiles(w):
        # [out, in] -> [m(out/128), 128(k-part), in/128 * 128(m-col)] so each
        # per-m weight block DMAs as one contiguous [128, K*128] copy
        o, i = w.shape
        t = w.reshape(o // 128, 128, i // 128, 128).transpose(0, 3, 2, 1)
        return np.ascontiguousarray(t.reshape(o // 128, 128, i))

    in_maps = []
    for core in range(NCORES):
        es = range(core * ELOC, (core + 1) * ELOC)
        xg = np.empty((ELOC, DIM, C), dtype=np.float32)
        for j, e in enumerate(es):
            np.take(xT_r, tok_pad[e], axis=1, out=xg[j])
        w1t = np.stack([wtiles(_round_tf32(w1[e])) for e in es])
        w3t = np.stack([wtiles(_round_tf32(w3[e])) for e in es])
        w2t = np.stack([wtiles(_round_tf32(w2[e])) for e in es])
        xs = np.ascontiguousarray(xT_r[:, core * TS:(core + 1) * TS])
        in_maps.append({
            "xg": xg.reshape(ELOC, KT, 128, C),
            "w1t": w1t, "w3t": w3t, "w2t": w2t,
            "xs": xs.reshape(KT, 128, TS),
            "sw1t": wtiles(_round_tf32(sw1)),
            "sw3t": wtiles(_round_tf32(sw3)),
            "sw2t": wtiles(_round_tf32(sw2)),
        })

    nc = _get_program(C, n_parts)
    res = run_bass_kernel_spmd(nc, in_maps, core_ids=list(range(NCORES)))

    # ---- host epilogue: combine-weight scatter-add + shared add ----
    y = np.zeros((T, DIM), dtype=np.float32)
    for core in range(NCORES):
        r = res.results[core]
        yg = r["yg"].reshape(ELOC, DIM, C)
        for j, e in enumerate(range(core * ELOC, (core + 1) * ELOC)):
            cnt = int(counts[e])
            if cnt == 0:
                continue
            toks = sorted_tok[offs[e]:offs[e + 1]]
            cw = sorted_w[offs[e]:offs[e + 1]]
            # toks are unique within one expert (top-k indices are distinct)
            y[toks] += cw[:, None] * yg[j, :, :cnt].T
        y[core * TS:(core + 1) * TS] += r["zs"].reshape(DIM, TS).T
    return y
